# revision 19
# baseline (speedup 1.0000x reference)
"""Trainium2 Bass kernel for nn_MultiHeadLayer (full-HB-axis multi-head attention).

Math (reference):
  q = queries @ W_Query; k = keys @ W_Key; v = values @ W_Value      [B, H*d]
  qh/kh/vh = split_heads(.)                                          [H*B, d]
  scores = (qh @ kh.T) / sqrt(d)   (FULL [HB, HB] matrix)
  att = softmax(scores, axis=-1);  out = merge_heads(att @ vh)       [B, H*d]

Sharding: row-parallel over the HB=16384 score rows; each of 8 cores owns 2048
contiguous rows (= one head-half: head m//2, batch half m%2) and computes its
[2048, HB] score slab flash-style. K/V projections are replicated per core
(cheap) instead of all-gathered.

Per-core kernel layout (all attention matmuls in bf16, f32 PSUM accum):
  S^T tiles [128 j, 512 i] = khT_jtile.T @ qhT  (row-packed pairs, K=64)
  exp on ScalarE (psum->sbuf, bf16), rowsum via a ones column in the V weights
  outT[e, i] += vh_aug_jt.T @ expS^T_jt  accumulated over 128 j-tiles in PSUM
"""

import numpy as np
import ml_dtypes

import concourse.bass as bass
import concourse.mybir as mybir
import concourse.tile as tile
from concourse import bacc, bass_utils

H = 4
D = 64          # head dim
E = 256         # embed
B = 4096
HB = H * B      # 16384
NCORES = 8
I = HB // NCORES  # 2048 q-rows per core
NIB = 4           # i-blocks per core
IBS = I // NIB    # 512
NJT = HB // 128   # 128 j-tiles
NJP = NJT // 2    # 64 j-pairs
SUP = 3           # S^T tiles per exp superstep (3 psum banks)

F32 = mybir.dt.float32
BF16 = mybir.dt.bfloat16
EXPF = mybir.ActivationFunctionType.Exp

_CACHE = {}


def _build_nc(dbg=False):
    nc = bacc.Bacc(
        "TRN2",
        target_bir_lowering=False,
        debug=False,
        enable_asserts=False,
        num_devices=NCORES,
    )
    qT = nc.dram_tensor("qT", [E, I], BF16, kind="ExternalInput").ap()
    kT = nc.dram_tensor("kT", [E, B], BF16, kind="ExternalInput").ap()
    vT = nc.dram_tensor("vT", [E, B], BF16, kind="ExternalInput").ap()
    wq = nc.dram_tensor("wq", [E, D], BF16, kind="ExternalInput").ap()
    wk = nc.dram_tensor("wk", [E, H * D], BF16, kind="ExternalInput").ap()
    wv = nc.dram_tensor("wv", [E, H * D], BF16, kind="ExternalInput").ap()
    outT = nc.dram_tensor("outT", [D, I], F32, kind="ExternalOutput").ap()
    rcp_d = [
        nc.dram_tensor(f"rcpd{ib}", [1, IBS], F32).ap() for ib in range(NIB)
    ]
    dbg_t = None
    if dbg:
        dbg_t = {
            "dbg_qh": nc.dram_tensor("dbg_qh", [128, I], BF16, kind="ExternalOutput").ap(),
            "dbg_kpair": nc.dram_tensor("dbg_kpair", [128, 8192], BF16, kind="ExternalOutput").ap(),
            "dbg_vh": nc.dram_tensor("dbg_vh", [128, NJT * 65], BF16, kind="ExternalOutput").ap(),
            "dbg_ex": nc.dram_tensor("dbg_ex", [128, SUP * 512], BF16, kind="ExternalOutput").ap(),
            "dbg_num": nc.dram_tensor("dbg_num", [65, IBS], F32, kind="ExternalOutput").ap(),
            "dbg_rcp": nc.dram_tensor("dbg_rcp", [1, IBS], F32, kind="ExternalOutput").ap(),
            "dbg_rbc": nc.dram_tensor("dbg_rbc", [64, IBS], F32, kind="ExternalOutput").ap(),
        }

    with tile.TileContext(nc) as tc:
        _kernel_body(nc, tc, qT, kT, vT, wq, wk, wv, outT, rcp_d, dbg_t)
    nc.compile()
    return nc


def _kernel_body(nc, tc, qT, kT, vT, wq, wk, wv, outT, rcp_d, dbg_t=None):
    with (
        tc.tile_pool(name="persist", bufs=1) as persist,
        tc.tile_pool(name="epil", bufs=2) as epil,
    ):
        # Persistent SBUF tensors for the main loop.
        qh = persist.tile([128, I], BF16, tag="qh")           # qhT/8, dup'd halves
        kpair = persist.tile([128, 64 * 128], BF16, tag="kpair")  # khT lo|hi halves
        vh65 = persist.tile([128, NJT, 65], BF16, tag="vh65")  # vh + ones col per jtile
        outsb = persist.tile([64, I], F32, tag="outsb")

        # ---------------- Phase A-D: DMA in + projections -----------------
        with (
            tc.tile_pool(name="stage", bufs=1) as stage,
            tc.tile_pool(name="phps", bufs=3, space="PSUM") as phps,
        ):
            wq_sb = stage.tile([128, 2, D], BF16, tag="wq")
            wk_sb = stage.tile([128, 2, H * D], BF16, tag="wk")
            wv_sb = stage.tile([128, 2, H * D], BF16, tag="wv")
            qT_sb = stage.tile([128, 2, I], BF16, tag="qT")
            kT_sb = stage.tile([128, 2, B], BF16, tag="kT")
            vT_sb = stage.tile([128, 2, B], BF16, tag="vT")

            nc.sync.dma_start(out=wq_sb, in_=wq.rearrange("(t p) m -> p t m", p=128))
            nc.sync.dma_start(out=qT_sb, in_=qT.rearrange("(t p) i -> p t i", p=128))
            nc.sync.dma_start(out=wk_sb, in_=wk.rearrange("(t p) m -> p t m", p=128))
            nc.sync.dma_start(out=kT_sb, in_=kT.rearrange("(t p) b -> p t b", p=128))
            nc.sync.dma_start(out=wv_sb, in_=wv.rearrange("(t p) m -> p t m", p=128))
            nc.sync.dma_start(out=vT_sb, in_=vT.rearrange("(t p) b -> p t b", p=128))

            # Phase B: qhT (scaled by 1/sqrt(d)=1/8), duplicated into both
            # partition halves (for row-packed MM1 pairs).
            for ib in range(NIB):
                ps_q = phps.tile([128, IBS], F32, tag="ph")
                isl = bass.ts(ib, IBS)
                for half in (0, 1):
                    for kt in (0, 1):
                        nc.tensor.matmul(
                            ps_q[half * 64:(half + 1) * 64, :],
                            lhsT=wq_sb[:, kt, :],
                            rhs=qT_sb[:, kt, isl],
                            start=(kt == 0),
                            stop=(kt == 1),
                        )
                nc.scalar.mul(qh[:, isl], ps_q[:, :], 0.125)

            # Phase C: khT -> kpair (partitions 0:64 = j-tiles 0..63, 64:128 =
            # j-tiles 64..127).
            for c in range(16):
                ps_k = phps.tile([128, 512], F32, tag="ph")
                for half in (0, 1):
                    j0 = half * 8192 + c * 512
                    h = j0 // B
                    b0 = j0 % B
                    for kt in (0, 1):
                        nc.tensor.matmul(
                            ps_k[half * 64:(half + 1) * 64, :],
                            lhsT=wk_sb[:, kt, h * D:(h + 1) * D],
                            rhs=kT_sb[:, kt, b0:b0 + 512],
                            start=(kt == 0),
                            stop=(kt == 1),
                        )
                nc.vector.tensor_copy(kpair[:, bass.ts(c, 512)], ps_k[:, :])

            # Phase D: vh per j-tile (+ ones column for the softmax rowsum).
            for bt in range(32):
                ps_v = phps.tile([128, H * D], F32, tag="ph")
                for kt in (0, 1):
                    nc.tensor.matmul(
                        ps_v[:, :],
                        lhsT=vT_sb[:, kt, bass.ts(bt, 128)],
                        rhs=wv_sb[:, kt, :],
                        start=(kt == 0),
                        stop=(kt == 1),
                    )
                vh4 = vh65.rearrange("p (h b) c -> p h b c", h=H)
                nc.vector.tensor_copy(
                    vh4[:, :, bt, 0:64],
                    ps_v.rearrange("p (h e) -> p h e", h=H),
                )
            nc.vector.memset(vh65[:, :, 64], 1.0)

        if dbg_t is not None:
            nc.sync.dma_start(out=dbg_t["dbg_qh"], in_=qh[:, :])
            nc.sync.dma_start(out=dbg_t["dbg_kpair"], in_=kpair[:, :])
            nc.sync.dma_start(
                out=dbg_t["dbg_vh"],
                in_=vh65.rearrange("p a b -> p (a b)"),
            )

        # ---------------- Main loop: flash attention over j ----------------
        with (
            tc.tile_pool(name="rps", bufs=2, space="PSUM") as rps,
            tc.tile_pool(name="rex", bufs=2) as rex,
            tc.tile_pool(name="ops", bufs=2, space="PSUM") as ops,
        ):
            for ib in range(NIB):
                isl = bass.ts(ib, IBS)
                ps_out = ops.tile([128, IBS], F32, tag="out")
                sup = {}  # superstep s -> [ps_tile, ex_tile, [(k, jt), ...]]

                def flush(s):
                    ps, ex, tiles = sup.pop(s)
                    n = len(tiles) * 512
                    nc.scalar.activation(ex[:, 0:n], ps[:, 0:n], EXPF)
                    if dbg_t is not None and ib == 0 and s == 0:
                        nc.sync.dma_start(out=dbg_t["dbg_ex"], in_=ex[:, :])
                    for k, jt in tiles:
                        off = k % SUP
                        nc.tensor.matmul(
                            ps_out[0:65, :],
                            lhsT=vh65[:, jt, :],
                            rhs=ex[:, off * 512:(off + 1) * 512],
                            start=(k == 0),
                            stop=(k == NJT - 1),
                        )

                for t in range(NJP):
                    for which in (0, 1):
                        k = 2 * t + which
                        jt = t if which == 0 else NJP + t
                        s = k // SUP
                        if s not in sup:
                            sup[s] = [
                                rps.tile([128, SUP * 512], F32, tag="ring",
                                         name="ring_ps"),
                                rex.tile([128, SUP * 512], BF16, tag="ring",
                                         name="ring_ex"),
                                [],
                            ]
                        p0, p1 = 64 * which, 64 * (which + 1)
                        nc.tensor.matmul(
                            sup[s][0][:, bass.ts(k % SUP, 512)],
                            lhsT=kpair[p0:p1, bass.ts(t, 128)],
                            rhs=qh[p0:p1, isl],
                            start=True,
                            stop=True,
                        )
                        sup[s][2].append((k, jt))
                    # flush every fully-populated superstep (keeps MM1 pairs
                    # adjacent in the PE stream)
                    for s in sorted(list(sup)):
                        if len(sup[s][2]) == SUP:
                            flush(s)
                for s in sorted(list(sup)):
                    flush(s)

                if dbg_t is not None and ib == 0:
                    dbg_num_sb = epil.tile([65, IBS], F32, tag="dbgnum",
                                           name="dbg_num_sb")
                    nc.vector.tensor_copy(dbg_num_sb, ps_out[0:65, :])
                    nc.sync.dma_start(out=dbg_t["dbg_num"], in_=dbg_num_sb)
                # Epilogue: normalize by the rowsum (psum row 64 of ps_out).
                # 1/rowsum on partition 64, bounce via DRAM to broadcast it
                # across partitions 0..63, then scale the numerators.
                rcp = epil.tile([65, IBS], F32, tag="rcp")
                nc.vector.reciprocal(rcp[64:65, :], ps_out[64:65, :])
                nc.sync.dma_start(out=rcp_d[ib], in_=rcp[64:65, :])
                rbc = epil.tile([64, IBS], F32, tag="rbc")
                nc.sync.dma_start(out=rbc, in_=rcp_d[ib].to_broadcast([64, IBS]))
                if dbg_t is not None and ib == 0:
                    nc.sync.dma_start(out=dbg_t["dbg_rcp"], in_=rcp[64:65, :])
                    nc.sync.dma_start(out=dbg_t["dbg_rbc"], in_=rbc)
                nc.vector.tensor_mul(outsb[:, isl], ps_out[0:64, :], rbc)
                nc.sync.dma_start(out=outT[:, isl], in_=outsb[:, isl])


def _get_nc():
    if "nc" not in _CACHE:
        _CACHE["nc"] = _build_nc()
    return _CACHE["nc"]


def _make_in_maps(queries, keys, values, W_Query, W_Key, W_Value):
    bf = ml_dtypes.bfloat16
    kTb = np.ascontiguousarray(np.asarray(keys, dtype=np.float32).T).astype(bf)
    vTb = np.ascontiguousarray(np.asarray(values, dtype=np.float32).T).astype(bf)
    wkb = np.ascontiguousarray(np.asarray(W_Key, dtype=np.float32)).astype(bf)
    wvb = np.ascontiguousarray(np.asarray(W_Value, dtype=np.float32)).astype(bf)
    qf = np.asarray(queries, dtype=np.float32)
    wqf = np.asarray(W_Query, dtype=np.float32)
    in_maps = []
    for m in range(NCORES):
        h, half = divmod(m, 2)
        b0 = half * I
        in_maps.append({
            "qT": np.ascontiguousarray(qf[b0:b0 + I].T).astype(bf),
            "kT": kTb,
            "vT": vTb,
            "wq": np.ascontiguousarray(wqf[:, h * D:(h + 1) * D]).astype(bf),
            "wk": wkb,
            "wv": wvb,
        })
    return in_maps


def _assemble(results):
    out = np.empty((B, H * D), np.float32)
    for m in range(NCORES):
        h, half = divmod(m, 2)
        b0 = half * I
        out[b0:b0 + I, h * D:(h + 1) * D] = results[m]["outT"].T
    return out


def kernel(queries, keys, values, W_Query, W_Key, W_Value):
    nc = _get_nc()
    in_maps = _make_in_maps(queries, keys, values, W_Query, W_Key, W_Value)
    res = bass_utils.run_bass_kernel_spmd(nc, in_maps, list(range(NCORES)))
    return _assemble(res.results)
